# revision 4
# baseline (speedup 1.0000x reference)
"""Causal single-head attention (B=4, S=2048, D=1024, fp32) on 8 TRN2 cores.

Sharding: core = (batch b, half h). Each core computes attention for 4
contiguous 256-query blocks of its batch, chosen so that both halves have
identical causal block structure (key-block counts [1,2,3,4] in local block
order), which keeps the SPMD instruction stream identical across cores:

    h=0 -> query blocks at rows {0, 768, 1024, 1792}
    h=1 -> query blocks at rows {256, 512, 1280, 1536}

Per-core device kernel (all matmuls in fp32r = FP22 mantissa, fp32 accum):
  qT = Wq^T x^T (queries gathered on host), kT = Wk^T x^T, v = x Wv
  (v round-trips through DRAM to bound SBUF residency).
  For each query block j (256 q), key blocks 0..j (512 keys each):
    scoresT[k, q] = kT^T q  (keys on partitions)
    += additive causal mask (host input, diagonal block only)
    attnT = exp(scoresT / 32)          (no max-subtraction: |s/32| <~ 6)
    AV[q, d] += attnT^T v, rowsum[q] += attnT^T ones   (PE matmuls)
  out[q, :] = AV[q, :] / rowsum[q]
"""

import numpy as np

B, S, D = 4, 2048, 1024
P = 128
NSL = D // P          # 8 contraction subtiles of 128
NQB = 4               # local query blocks (256 queries each)
QB_MAP = {0: (0, 768, 1024, 1792), 1: (256, 512, 1280, 1536)}
MASK_NEG = -1.0e9
SCALE = 1.0 / np.sqrt(np.float32(D))

_CACHE = {}


def _build_nc(reps=1):
    from contextlib import ExitStack

    import concourse.mybir as mybir
    from concourse import bacc
    from concourse.tile import TileContext

    f32 = mybir.dt.float32
    f32r = mybir.dt.float32r
    Exp = mybir.ActivationFunctionType.Exp

    nc = bacc.Bacc("TRN2", target_bir_lowering=False, debug=False,
                   enable_asserts=False, num_devices=8)
    xT_d = nc.dram_tensor("xT", [D, S], f32r, kind="ExternalInput").ap()
    xTq_d = nc.dram_tensor("xTq", [D, 1024], f32r, kind="ExternalInput").ap()
    wq_d = nc.dram_tensor("wq", [D, D], f32r, kind="ExternalInput").ap()
    wk_d = nc.dram_tensor("wk", [D, D], f32r, kind="ExternalInput").ap()
    wv_d = nc.dram_tensor("wv", [D, D], f32r, kind="ExternalInput").ap()
    mask_d = nc.dram_tensor("mask", [NQB, 2, P, 2, 256], f32,
                            kind="ExternalInput").ap()
    ones_d = nc.dram_tensor("onesd", [P, 2], f32r, kind="ExternalInput").ap()
    out_d = nc.dram_tensor("out", [1024, D], f32, kind="ExternalOutput").ap()

    with TileContext(nc) as tc, ExitStack() as ctx:
        persist = ctx.enter_context(tc.tile_pool(name="persist", bufs=1))
        qT = persist.tile([P, NSL, 1024], f32r, tag="qT", name="qT")
        kT = persist.tile([P, NSL, 2048], f32r, tag="kT", name="kT")
        ones = persist.tile([P, 2], f32r, tag="ones", name="ones")
        nc.sync.dma_start(out=ones, in_=ones_d)
        dram = ctx.enter_context(tc.tile_pool(name="dram", bufs=1, space="DRAM"))
        v_dram = dram.tile([16, P, 1024], f32r, tag="vd", name="v_dram")

        for _rep in range(reps):
            _build_body(nc, tc, mybir, f32, f32r, Exp, qT, kT, ones, v_dram,
                        xT_d, xTq_d, wq_d, wk_d, wv_d, mask_d, out_d)
    nc.compile()
    return nc


def _build_body(nc, tc, mybir, f32, f32r, Exp, qT, kT, ones, v_dram,
                xT_d, xTq_d, wq_d, wk_d, wv_d, mask_d, out_d):
    if True:
        # ---- Q projection: qT[d_out, q] = Wq^T @ x^T ----
        with tc.tile_pool(name="pq", bufs=1) as pq, \
             tc.tile_pool(name="psq", bufs=4, space="PSUM") as psq:
            w = pq.tile([P, NSL, 1024], f32r, tag="w", name="wq_t")
            nc.sync.dma_start(out=w, in_=wq_d.rearrange("(s p) o -> p s o", p=P))
            xq = pq.tile([P, NSL, 1024], f32r, tag="xq", name="xq_t")
            nc.sync.dma_start(out=xq, in_=xTq_d.rearrange("(s p) q -> p s q", p=P))
            for c in range(NSL):
                for g in range(2):
                    ps = psq.tile([P, 512], f32, tag="ps", name="ps_q")
                    for s in range(NSL):
                        nc.tensor.matmul(ps, (w[:, s, c * P:(c + 1) * P]),
                                         (xq[:, s, g * 512:(g + 1) * 512]),
                                         start=(s == 0), stop=(s == NSL - 1))
                    nc.vector.tensor_copy(out=qT[:, c, g * 512:(g + 1) * 512],
                                          in_=ps)

        # ---- K projection: kT[d_out, k] = Wk^T @ x^T ----
        with tc.tile_pool(name="pk", bufs=1) as pk, \
             tc.tile_pool(name="pkx", bufs=2) as pkx, \
             tc.tile_pool(name="psk", bufs=4, space="PSUM") as psk:
            w = pk.tile([P, NSL, 1024], f32r, tag="w", name="wk_t")
            nc.sync.dma_start(out=w, in_=wk_d.rearrange("(s p) o -> p s o", p=P))
            xT_r = xT_d.rearrange("(s p) k -> p s k", p=P)
            for g in range(4):
                xg = pkx.tile([P, NSL, 512], f32r, tag="xg", name="xg_t")
                nc.sync.dma_start(out=xg, in_=xT_r[:, :, g * 512:(g + 1) * 512])
                for c in range(NSL):
                    ps = psk.tile([P, 512], f32, tag="ps", name="ps_k")
                    for s in range(NSL):
                        nc.tensor.matmul(ps, (w[:, s, c * P:(c + 1) * P]),
                                         (xg[:, s, :]),
                                         start=(s == 0), stop=(s == NSL - 1))
                    nc.vector.tensor_copy(out=kT[:, c, g * 512:(g + 1) * 512],
                                          in_=ps)

        # ---- V projection: v[k, d_out] = x @ Wv (spilled to DRAM) ----
        with tc.tile_pool(name="pv", bufs=1) as pv, \
             tc.tile_pool(name="pvx", bufs=3) as pvx, \
             tc.tile_pool(name="pvo", bufs=3) as pvo, \
             tc.tile_pool(name="psv", bufs=4, space="PSUM") as psv:
            w = pv.tile([P, NSL, 1024], f32r, tag="w", name="wv_t")
            nc.sync.dma_start(out=w, in_=wv_d.rearrange("(s p) o -> p s o", p=P))
            xT_r = xT_d.rearrange("(s p) k -> p s k", p=P)
            for kt in range(16):
                xk = pvx.tile([P, NSL, P], f32r, tag="xk", name="xk_t")
                nc.sync.dma_start(out=xk, in_=xT_r[:, :, kt * P:(kt + 1) * P])
                vo = pvo.tile([P, 1024], f32r, tag="vo", name="vo_t")
                for hh in range(2):
                    ps = psv.tile([P, 512], f32, tag="ps", name="ps_v")
                    for s in range(NSL):
                        nc.tensor.matmul(ps, (xk[:, s, :]),
                                         (w[:, s, hh * 512:(hh + 1) * 512]),
                                         start=(s == 0), stop=(s == NSL - 1))
                    nc.scalar.copy(out=vo[:, hh * 512:(hh + 1) * 512], in_=ps)
                nc.sync.dma_start(out=v_dram[kt], in_=vo)

        # ---- Attention ----
        with tc.tile_pool(name="avv", bufs=3) as vvp, \
             tc.tile_pool(name="aat", bufs=3) as atp, \
             tc.tile_pool(name="amk", bufs=2) as mkp, \
             tc.tile_pool(name="aot", bufs=2) as otp, \
             tc.tile_pool(name="asm", bufs=4) as smp, \
             tc.tile_pool(name="pssc", bufs=2, space="PSUM") as pssc, \
             tc.tile_pool(name="psav", bufs=4, space="PSUM") as psav, \
             tc.tile_pool(name="pssm", bufs=2, space="PSUM") as pssm:
            for j in range(NQB):
                npair = 2 * (j + 1)      # 256-key pairs: key blocks 0..j
                av = [psav.tile([P, 512], f32, tag="av", name=f"av_{j}_{i}")
                      for i in range(4)]             # [qsub*2 + dhalf]
                sums = [pssm.tile([P, 2], f32, tag="sums", name=f"sums_{j}_{qs}")
                        for qs in range(2)]
                for pr in range(npair):
                    sc = pssc.tile([P, 2, 256], f32, tag="sc", name=f"sc_{j}_{pr}")
                    for t in range(2):
                        ktile = 2 * pr + t
                        for s in range(NSL):
                            nc.tensor.matmul(
                                sc[:, t, :],
                                (kT[:, s, ktile * P:(ktile + 1) * P]),
                                (qT[:, s, j * 256:(j + 1) * 256]),
                                start=(s == 0), stop=(s == NSL - 1))
                    if pr >= npair - 2:  # diagonal 512-key block: apply mask
                        mk = mkp.tile([P, 2, 256], f32, tag="mk",
                                      name=f"mk_{j}_{pr}")
                        nc.sync.dma_start(out=mk, in_=mask_d[j, pr - (npair - 2)])
                        nc.vector.tensor_add(out=sc, in0=sc, in1=mk)
                    at = atp.tile([P, 2, 256], f32r, tag="at", name=f"at_{j}_{pr}")
                    nc.scalar.activation(out=at, in_=sc, func=Exp,
                                         scale=float(SCALE))
                    for t in range(2):
                        ktile = 2 * pr + t
                        vv = vvp.tile([P, 1024], f32r, tag="vv",
                                      name=f"vv_{j}_{pr}_{t}")
                        nc.sync.dma_start(out=vv, in_=v_dram[ktile])
                        st = (pr == 0 and t == 0)
                        sp = (pr == npair - 1 and t == 1)
                        for qs in range(2):
                            lhs = (at[:, t, qs * P:(qs + 1) * P])
                            for hh in range(2):
                                nc.tensor.matmul(av[qs * 2 + hh], lhs,
                                                 (vv[:, hh * 512:(hh + 1) * 512]),
                                                 start=st, stop=sp)
                            nc.tensor.matmul(sums[qs], lhs, ones,
                                             start=st, stop=sp)
                for qs in range(2):
                    rec = smp.tile([P, 1], f32, tag="rec", name=f"rec_{j}_{qs}")
                    nc.vector.reciprocal(out=rec, in_=sums[qs][:, 0:1])
                    ot = otp.tile([P, 1024], f32, tag="ot", name=f"ot_{j}_{qs}")
                    for hh in range(2):
                        nc.vector.tensor_scalar_mul(ot[:, hh * 512:(hh + 1) * 512],
                                                    av[qs * 2 + hh], rec)
                    row = j * 256 + qs * P
                    nc.sync.dma_start(out=out_d[row:row + P, :], in_=ot)


def _make_mask(h):
    m = np.zeros((NQB, 2, P, 2, 256), np.float32)
    for j, qb in enumerate(QB_MAP[h]):
        for prr in range(2):
            for t in range(2):
                keys = 512 * j + 256 * prr + 128 * t + np.arange(P)
                qs_ = qb + np.arange(256)
                m[j, prr, :, t, :] = np.where(keys[:, None] <= qs_[None, :],
                                              0.0, MASK_NEG)
    return m


def build_in_maps(x, Wq, Wk, Wv):
    x = np.ascontiguousarray(np.asarray(x, dtype=np.float32))
    Wq = np.ascontiguousarray(np.asarray(Wq, dtype=np.float32))
    Wk = np.ascontiguousarray(np.asarray(Wk, dtype=np.float32))
    Wv = np.ascontiguousarray(np.asarray(Wv, dtype=np.float32))
    masks = {h: _make_mask(h) for h in (0, 1)}
    in_maps = []
    for b in range(B):
        xt = np.ascontiguousarray(x[b].T)
        for h in range(2):
            xtq = np.ascontiguousarray(np.concatenate(
                [xt[:, qb:qb + 256] for qb in QB_MAP[h]], axis=1))
            in_maps.append({"xT": xt, "xTq": xtq, "wq": Wq, "wk": Wk,
                            "wv": Wv, "mask": masks[h],
                            "onesd": np.ones((P, 2), np.float32)})
    return in_maps


def assemble_out(results):
    out = np.empty((B, S, D), np.float32)
    for b in range(B):
        for h in range(2):
            o = results[2 * b + h]["out"]
            for ji, qb in enumerate(QB_MAP[h]):
                out[b, qb:qb + 256] = o[ji * 256:(ji + 1) * 256]
    return out


def get_nc():
    nc = _CACHE.get("nc")
    if nc is None:
        nc = _build_nc()
        _CACHE["nc"] = nc
    return nc


def kernel(x, Wq, Wk, Wv):
    from concourse.bass_utils import run_bass_kernel_spmd

    nc = get_nc()
    in_maps = build_in_maps(x, Wq, Wk, Wv)
    res = run_bass_kernel_spmd(nc, in_maps, core_ids=list(range(8)))
    return assemble_out(res.results)
